# revision 1
# baseline (speedup 1.0000x reference)
"""FAVOR+ (Performer) multi-head causal attention — Trainium2 Bass kernel.

Sharding: 8 cores = 4 batches x 2 head-groups (4 heads each).
Math note: the softmax-kernel stabilizers (stab) and the +eps term only
rescale qp/kp per (l,h) [or globally], which cancels in num/den up to an
O(eps * e^stab) ~ 3e-4 relative perturbation (verified numerically).
We therefore compute raw exp(dd) for Q (diag_q also cancels per-l) and
exp(dd - diag_k) for K, with no eps and no stabilizers -> no collectives.

All matmuls run as float32r (full-rate fp32 streaming mode, needs N>=256
and engine-rounded operands).

Per-core pipeline:
  1. qT,kT = proj(x) in [c=256, L] layout (+bias), v in chunked [L, c]
     layout with an appended ones column per head (fused den/s_k rows).
  2. Per head: qpT/kpT [m, L] feature maps (diag_k via augmented matmul),
     kp_lc [L, m] copy for the state update (diag_k via Exp bias).
  3. Causal chunked scan, C=256: A^T blocks, masked; numden^T[65, C] =
     v_aug.T @ A^T + S_aug.T @ qpT_c ; attnT = num * bcast(1/den);
     S_aug += kp_lc.T @ v_aug.
  4. outT[512, L] = wo.T @ attnT (partial; host sums the 2 head-groups).
"""
import numpy as np

B, L, DIM, H, DK, M = 4, 2048, 512, 8, 64, 256
HPC = 4            # heads per core
CW = 256           # scan chunk width
NC2 = L // 128     # 16
NCC = L // CW      # 8
LT = 512
NLT = L // LT

_COMPILED = None


def _build():
    import concourse.bacc as bacc
    import concourse.mybir as mybir
    from concourse.tile import TileContext

    f32 = mybir.dt.float32
    f32r = mybir.dt.float32r
    EXP = mybir.ActivationFunctionType.Exp

    nc = bacc.Bacc("TRN2", target_bir_lowering=False, debug=False,
                   enable_asserts=False, num_devices=8)

    def din(name, shape):
        return nc.dram_tensor(name, shape, f32, kind="ExternalInput").ap()

    xq = din("xq", [512, L])
    xk = din("xk", [512, L])
    xv = din("xv", [512, L])
    wq = din("wq", [512, 256])
    wk = din("wk", [512, 256])
    wv = din("wv", [513, 256])      # [Wv_slice.T ; bv]
    bq2 = din("bq2", [128, 2])
    bk2 = din("bk2", [128, 2])
    projT = din("projT", [128, 256])  # proj.T stacked twice
    bd = din("bd", [128, 8])
    msk = din("msk", [128, 512])
    wo = din("wo", [256, 512])
    outT = nc.dram_tensor("outT", [512, L], f32, kind="ExternalOutput").ap()
    ksr_d = nc.dram_tensor("ksr_d", [1, HPC * L], f32, kind="Internal").ap()

    with TileContext(nc) as tc:
        with (
            tc.tile_pool(name="const", bufs=1) as cpool,
            tc.tile_pool(name="persist", bufs=1) as ppool,
            tc.tile_pool(name="psF", bufs=2, space="PSUM") as psF,
        ):
            # ---- constants (DMA to f32 staging, engine-round into f32r) ----
            def ldconst(name, shape, src):
                tmp = cpool.tile(shape, f32, name=name + "_s", tag="cstage",
                                 bufs=2)
                nc.sync.dma_start(tmp[:, :], src)
                t = cpool.tile(shape, f32r, name=name, tag=name)
                nc.any.tensor_copy(t[:, :], tmp[:, :])
                return t

            c_projT = ldconst("projT", [128, 256], projT)
            c_bd = ldconst("bd", [128, 8], bd)
            c_wq = [ldconst(f"wq{i}", [128, 256], wq[128 * i:128 * (i + 1), :])
                    for i in range(4)]
            c_wk = [ldconst(f"wk{i}", [128, 256], wk[128 * i:128 * (i + 1), :])
                    for i in range(4)]
            c_wv = [ldconst(f"wv{i}", [128, 256], wv[128 * i:128 * (i + 1), :])
                    for i in range(4)]
            c_wvb = ldconst("wvb", [1, 256], wv[512:513, :])
            c_wo = [ldconst(f"wo{i}", [64, 512], wo[64 * i:64 * (i + 1), :])
                    for i in range(4)]
            c_msk = cpool.tile([128, 512], f32, tag="msk")
            nc.sync.dma_start(c_msk[:, :], msk)
            c_bq = cpool.tile([128, 2], f32, tag="bq")
            nc.sync.dma_start(c_bq[:, :], bq2)
            c_bk = cpool.tile([128, 2], f32, tag="bk")
            nc.sync.dma_start(c_bk[:, :], bk2)
            c_ones32 = cpool.tile([128, 128], f32, tag="ones32")
            nc.any.memset(c_ones32[:, :], 1.0)
            c_ones = cpool.tile([128, 128], f32r, tag="ones")
            nc.any.tensor_copy(c_ones[:, :], c_ones32[:, :])
            c_zero32 = cpool.tile([128, 132], f32, tag="zero32")
            nc.any.memset(c_zero32[:, :], 0.0)

            # persistent activations
            t_qT = [ppool.tile([128, L], f32r, name=f"qT{i}", tag=f"qT{i}")
                    for i in range(2)]
            t_kT = [ppool.tile([128, L], f32r, name=f"kT{i}", tag=f"kT{i}")
                    for i in range(2)]
            t_v = ppool.tile([128, NC2 * 264], f32r, tag="vall")
            t_ksc = ppool.tile([128, NC2 * 4], f32, tag="ksc")

            # ---- Phase 1: projections (x staged + rounded, then released) ----
            xin = tc.tile_pool(name="xin", bufs=1)
            xpool = xin.__enter__()
            t_x = {}
            for nm, src in (("q", xq), ("k", xk), ("v", xv)):
                for i in range(4):
                    tmp = xpool.tile([128, L], f32, name=f"xs{nm}{i}",
                                     tag="xstage", bufs=2)
                    nc.sync.dma_start(tmp[:, :], src[128 * i:128 * (i + 1), :])
                    xt = xpool.tile([128, L], f32r, name=f"x{nm}{i}",
                                    tag=f"x{nm}{i}")
                    nc.any.tensor_copy(xt[:, :], tmp[:, :])
                    t_x[(nm, i)] = xt

            for half in range(2):
                for lt in range(NLT):
                    ls = slice(lt * LT, (lt + 1) * LT)
                    for (wgt, nm, dst, bias) in ((c_wq, "q", t_qT, c_bq),
                                                 (c_wk, "k", t_kT, c_bk)):
                        ps = psF.tile([128, LT], f32, tag="psF")
                        for kt in range(4):
                            nc.tensor.matmul(
                                ps[:, :],
                                wgt[kt][:, 128 * half:128 * (half + 1)],
                                t_x[(nm, kt)][:, ls],
                                start=(kt == 0), stop=(kt == 3))
                        nc.vector.tensor_scalar_add(
                            dst[half][:, ls], ps[:, :], bias[:, half:half + 1])

            v_r = t_v[:, :].rearrange("p (c x) -> p c x", x=66)
            nc.any.tensor_copy(v_r[:, :, 64:66], c_ones32[:, 0:128])
            for ch in range(NC2):
                cs = slice(ch * 128, (ch + 1) * 128)
                ps = psF.tile([128, 256], f32, tag="psF")
                for kt in range(4):
                    nc.tensor.matmul(ps[:, :], t_x[("v", kt)][:, cs],
                                     c_wv[kt][:, :],
                                     start=(kt == 0), stop=False)
                nc.tensor.matmul(ps[:, :], c_ones[0:1, 0:128],
                                 c_wvb[:, :], start=False, stop=True)
                for h in range(HPC):
                    nc.any.tensor_copy(
                        t_v[:, ch * 264 + h * 66:ch * 264 + h * 66 + 64],
                        ps[:, 64 * h:64 * (h + 1)])
            xin.__exit__(None, None, None)

            # ---- Phase 1.5: -diag_k (row layout -> DRAM, column layout) ----
            sqx = tc.tile_pool(name="sqx", bufs=1)
            sqpool = sqx.__enter__()
            t_sq = [sqpool.tile([128, L], f32r, name=f"sq{i}", tag=f"sq{i}")
                    for i in range(2)]
            for half in range(2):
                nc.vector.tensor_mul(t_sq[half][:, :], t_kT[half][:, :],
                                     t_kT[half][:, :])
            for h4 in range(HPC):
                for lt in range(NLT):
                    ls = slice(lt * LT, (lt + 1) * LT)
                    ps = psF.tile([1, LT], f32, tag="psF")
                    for half in range(2):
                        nc.tensor.matmul(
                            ps[:, :],
                            c_bd[:, 4 * half + h4:4 * half + h4 + 1],
                            t_sq[half][:, ls],
                            start=(half == 0), stop=(half == 1))
                    t_ksrt = sqpool.tile([1, LT], f32, tag="ksrt", bufs=2)
                    nc.any.tensor_copy(t_ksrt[:, :], ps[:, :])
                    nc.sync.dma_start(
                        ksr_d[0:1, h4 * L + lt * LT:h4 * L + (lt + 1) * LT],
                        t_ksrt[:, :])
            for ch in range(NC2):
                cs = slice(ch * 128, (ch + 1) * 128)
                ps = psF.tile([128, 4], f32, tag="psF")
                for half in range(2):
                    nc.tensor.matmul(ps[:, :], t_sq[half][:, cs],
                                     c_bd[:, 4 * half:4 * (half + 1)],
                                     start=(half == 0), stop=(half == 1))
                nc.any.tensor_copy(t_ksc[:, 4 * ch:4 * (ch + 1)], ps[:, :])
            sqx.__exit__(None, None, None)

            # ---- Phase 2+3: per head ----
            actx = tc.tile_pool(name="attn", bufs=1)
            apool = actx.__enter__()
            hctx = (tc.tile_pool(name="headbuf", bufs=2),
                    tc.tile_pool(name="headbuf1", bufs=1),
                    tc.tile_pool(name="work", bufs=3),
                    tc.tile_pool(name="psScan", bufs=1, space="PSUM"),
                    tc.tile_pool(name="psND", bufs=2, space="PSUM"))
            hpool, h1pool, wpool, psS_pool, psND_pool = [
                c.__enter__() for c in hctx]
            t_attnT = [apool.tile([64, L], f32r, name=f"attnT{i}",
                                  tag=f"attnT{i}") for i in range(4)]
            for h in range(HPC):
                hh = h // 2
                hr = slice(64 * (h % 2), 64 * (h % 2) + 64)
                pr = slice(64 * (h % 2), 64 * (h % 2) + 64)
                t_qp = [hpool.tile([128, L], f32r, name=f"qp{i}", tag=f"qp{i}")
                        for i in range(2)]
                t_kp = [h1pool.tile([128, L], f32r, name=f"kp{i}",
                                    tag=f"kp{i}") for i in range(2)]
                for lt in range(NLT):
                    ls = slice(lt * LT, (lt + 1) * LT)
                    t_ksrh0 = hpool.tile([1, LT], f32, tag="ksrh0")
                    nc.sync.dma_start(
                        t_ksrh0[:, :],
                        ksr_d[0:1, h * L + lt * LT:h * L + (lt + 1) * LT])
                    t_ksrh = hpool.tile([1, LT], f32r, tag="ksrh")
                    nc.any.tensor_copy(t_ksrh[:, :], t_ksrh0[:, :])
                    for half in range(2):
                        mh = slice(128 * half, 128 * (half + 1))
                        ps = psF.tile([128, LT], f32, tag="psF")
                        nc.tensor.matmul(ps[:, :], c_projT[pr, mh],
                                         t_qT[hh][hr, ls],
                                         start=True, stop=True)
                        nc.scalar.activation(t_qp[half][:, ls], ps[:, :], EXP)
                        ps2 = psF.tile([128, LT], f32, tag="psF")
                        nc.tensor.matmul(ps2[:, :], c_projT[pr, mh],
                                         t_kT[hh][hr, ls],
                                         start=True, stop=False)
                        nc.tensor.matmul(ps2[:, :], c_ones[0:1, 0:128],
                                         t_ksrh[0:1, :],
                                         start=False, stop=True)
                        nc.scalar.activation(t_kp[half][:, ls], ps2[:, :], EXP)
                t_kplc = h1pool.tile([128, NC2 * 256], f32r, tag="kplc")
                for ch in range(NC2):
                    cs = slice(ch * 128, (ch + 1) * 128)
                    ps = psF.tile([128, 256], f32, tag="psF")
                    nc.tensor.matmul(ps[:, :], t_kT[hh][hr, cs],
                                     c_projT[pr, :], start=True, stop=True)
                    nc.scalar.activation(
                        t_kplc[:, 256 * ch:256 * (ch + 1)], ps[:, :], EXP,
                        bias=t_ksc[:, 4 * ch + h:4 * ch + h + 1])

                # scan
                t_S = h1pool.tile([128, 132], f32r, tag="S")
                nc.any.tensor_copy(t_S[:, :], c_zero32[:, :])
                t_den = h1pool.tile([1, L], f32, tag="den")
                for cc in range(NCC):
                    qs = slice(cc * CW, (cc + 1) * CW)
                    ts0 = slice(cc * CW, cc * CW + 128)
                    ts1 = slice(cc * CW + 128, (cc + 1) * CW)
                    psA = psS_pool.tile([128, 512], f32, tag="psA", bufs=2)
                    nc.tensor.matmul(psA[:, 0:256], t_kp[0][:, ts0],
                                     t_qp[0][:, qs], start=True, stop=False)
                    nc.tensor.matmul(psA[:, 0:256], t_kp[1][:, ts0],
                                     t_qp[1][:, qs], start=False, stop=False)
                    nc.tensor.matmul(psA[:, 256:512], t_kp[0][:, ts1],
                                     t_qp[0][:, qs], start=False, stop=False)
                    nc.tensor.matmul(psA[:, 256:512], t_kp[1][:, ts1],
                                     t_qp[1][:, qs], start=False, stop=True)
                    atm = wpool.tile([128, 512], f32r, tag="atm")
                    nc.vector.tensor_mul(atm[:, :], psA[:, :], c_msk[:, :])
                    nd = psND_pool.tile([66, CW], f32, tag="psNDt")
                    c128 = cc * 2
                    va0 = t_v[:, c128 * 264 + h * 66:c128 * 264 + h * 66 + 66]
                    va1 = t_v[:, (c128 + 1) * 264 + h * 66:
                              (c128 + 1) * 264 + h * 66 + 66]
                    nc.tensor.matmul(nd[:, :], va0, atm[:, 0:256],
                                     start=True, stop=False)
                    nc.tensor.matmul(nd[:, :], va1, atm[:, 256:512],
                                     start=False, stop=False)
                    nc.tensor.matmul(nd[:, :], t_S[:, 0:66], t_qp[0][:, qs],
                                     start=False, stop=False)
                    nc.tensor.matmul(nd[:, :], t_S[:, 66:132], t_qp[1][:, qs],
                                     start=False, stop=True)
                    nc.any.tensor_copy(t_attnT[h][:, qs], nd[0:64, :])
                    nc.any.tensor_copy(t_den[0:1, qs], nd[64:65, :])
                    psS = psS_pool.tile([128, 132], f32, tag="psS")
                    nc.tensor.matmul(
                        psS[:, 0:66],
                        t_kplc[:, c128 * 256:c128 * 256 + 128],
                        va0, start=True, stop=False)
                    nc.tensor.matmul(
                        psS[:, 0:66],
                        t_kplc[:, (c128 + 1) * 256:(c128 + 1) * 256 + 128],
                        va1, start=False, stop=False)
                    nc.tensor.matmul(
                        psS[:, 66:132],
                        t_kplc[:, c128 * 256 + 128:c128 * 256 + 256],
                        va0, start=False, stop=False)
                    nc.tensor.matmul(
                        psS[:, 66:132],
                        t_kplc[:, (c128 + 1) * 256 + 128:(c128 + 2) * 256],
                        va1, start=False, stop=True)
                    with nc.allow_low_precision(reason="f32r state accumulate (TF32-rounding ~1e-3, validated vs reference)"):
                        nc.vector.tensor_add(t_S[:, :], t_S[:, :], psS[:, :])
                # division for the whole head, off the chunk chain
                t_rcpr = h1pool.tile([1, L], f32r, tag="rcpr")
                with nc.allow_low_precision(reason="f32r reciprocal for matmul broadcast (validated vs reference)"):
                    nc.vector.reciprocal(t_rcpr[0:1, :], t_den[0:1, :])
                for lt in range(NLT):
                    ls = slice(lt * LT, (lt + 1) * LT)
                    psB = psF.tile([64, LT], f32, name="psB", tag="psF")
                    nc.tensor.matmul(psB[:, :], c_ones[0:1, 0:64],
                                     t_rcpr[0:1, ls], start=True, stop=True)
                    nc.vector.tensor_mul(t_attnT[h][:, ls], t_attnT[h][:, ls],
                                         psB[:, :])
            for c in reversed(hctx):
                c.__exit__(None, None, None)

            # ---- Phase 4: output projection ----
            octx = tc.tile_pool(name="outp", bufs=2)
            opool = octx.__enter__()
            for osub in range(4):
                os_ = slice(128 * osub, 128 * (osub + 1))
                t_o = opool.tile([128, L], f32, tag="outT")
                for lt in range(NLT):
                    ls = slice(lt * LT, (lt + 1) * LT)
                    ps = psF.tile([128, LT], f32, tag="psF")
                    for h in range(4):
                        nc.tensor.matmul(ps[:, :], c_wo[h][:, os_],
                                         t_attnT[h][:, ls],
                                         start=(h == 0), stop=(h == 3))
                    nc.any.tensor_copy(t_o[:, ls], ps[:, :])
                nc.sync.dma_start(outT[os_, :], t_o[:, :])
            octx.__exit__(None, None, None)
            actx.__exit__(None, None, None)

    nc.compile()
    return nc


def _prep_inputs(query, key, value, Wq, bq, Wk, bk, Wv, bv, Wo, bo, proj):
    s = float(DK) ** -0.25
    tri = (np.arange(128)[:, None] <= np.arange(128)[None, :]).astype(np.float32)
    on = np.ones((128, 128), np.float32)
    zr = np.zeros((128, 128), np.float32)
    msk = np.concatenate([tri, on, zr, tri], axis=1)
    bd = np.zeros((128, 8), np.float32)
    for half in range(2):
        for r in range(128):
            bd[r, 4 * half + (2 * half + r // 64)] = -0.5
    pT = np.ascontiguousarray(proj.T)
    common = {"projT": np.concatenate([pT, pT]), "bd": bd, "msk": msk}
    in_maps = []
    for b in range(B):
        for hg in range(2):
            sl = slice(hg * 256, (hg + 1) * 256)
            Wqs, Wks, Wvs = Wq[sl] * s, Wk[sl] * s, Wv[sl]
            bqs, bks, bvs = bq[sl] * s, bk[sl] * s, bv[sl]
            m = dict(common)
            m["xq"] = np.ascontiguousarray(query[b].T)
            m["xk"] = np.ascontiguousarray(key[b].T)
            m["xv"] = np.ascontiguousarray(value[b].T)
            m["wq"] = np.ascontiguousarray(Wqs.T)
            m["wk"] = np.ascontiguousarray(Wks.T)
            m["wv"] = np.concatenate([Wvs.T, bvs[None, :]])
            m["bq2"] = np.stack([bqs[:128], bqs[128:]], axis=1)
            m["bk2"] = np.stack([bks[:128], bks[128:]], axis=1)
            m["wo"] = np.ascontiguousarray(Wo[:, sl].T)
            in_maps.append({k: np.ascontiguousarray(v, np.float32)
                            for k, v in m.items()})
    return in_maps


def kernel(query, key, value, Wq, bq, Wk, bk, Wv, bv, Wo, bo, proj,
           _trace=False):
    global _COMPILED
    from concourse import bass_utils
    args = [np.asarray(a, np.float32) for a in
            (query, key, value, Wq, bq, Wk, bk, Wv, bv, Wo, bo, proj)]
    if _COMPILED is None:
        _COMPILED = _build()
    in_maps = _prep_inputs(*args)
    res = bass_utils.run_bass_kernel_spmd(
        _COMPILED, in_maps, core_ids=list(range(8)), trace=_trace)
    out = np.empty((B, L, DIM), np.float32)
    bo_ = args[10]
    for b in range(B):
        out[b] = (res.results[2 * b]["outT"].T
                  + res.results[2 * b + 1]["outT"].T + bo_)
    if _trace:
        kernel._last = res
    return out



# revision 25
# speedup vs baseline: 1.4384x; 1.4384x over previous
"""FAVOR+ (Performer) multi-head causal attention — Trainium2 Bass kernel v2.

Sharding: 8 cores = 4 batches x 2 head-groups (4 heads each).

Math note: the softmax-kernel stabilizers and +eps only rescale qp/kp per
(l,h) [or globally] and cancel in num/den (O(3e-4) perturbation). We compute
raw exp(dd) for Q and exp(dd - diag_k) for K; no collectives needed.

v2 changes vs v1:
  * All scan-side math in bf16 (PSUM accumulates f32). Host ships x and
    weights pre-converted to bf16: no on-chip rounding copies, half the DMA.
    Validated numerically: 4.8e-3 rel err vs f32 reference (tol 2e-2).
  * Hierarchical scan: all chunk state-sums psS_cc = kplc^T v_aug run first
    and are prefix-added into per-chunk S snapshots, making every chunk's
    output matmuls independent (no serial chunk chain).
  * diag_k folded into the kp matmul as an accumulating (-0.5)-matmul on
    sq = kT^2 (kills v1's DRAM round-trip for the stabilizer row).
  * A-blocks skip the always-zero (key>query-block) quarter: psA is
    [128, 384] = [keys-lo x 256q | keys-hi x 128q-hi].
  * attnT packed 2 heads per [128, L] tile -> wo runs 2x128-contract.
  * den: per-chunk reciprocal + PE ones-broadcast, divide fused into the
    PSUM->SBUF copy of num.
"""
import numpy as np

B, L, DIM, H, DK, M = 4, 2048, 512, 8, 64, 256
HPC = 4            # heads per core
CW = 256           # scan chunk width (queries per chunk)
NCC = L // CW      # 8
NC2 = L // 128     # 16
LT = 512
NLT = L // LT

_COMPILED = None
_DEBUG_ATT = False


def _build():
    import concourse.bacc as bacc
    import concourse.mybir as mybir
    from concourse.tile import TileContext

    f32 = mybir.dt.float32
    bf16 = mybir.dt.bfloat16
    EXP = mybir.ActivationFunctionType.Exp

    nc = bacc.Bacc("TRN2", target_bir_lowering=False, debug=False,
                   enable_asserts=False, num_devices=8)

    def din(name, shape, dt=bf16):
        return nc.dram_tensor(name, shape, dt, kind="ExternalInput").ap()

    xq = din("xq", [512, L])
    xk = din("xk", [512, L])
    xv = din("xv", [512, L])
    wq = din("wq", [512, 256])
    wk = din("wk", [512, 256])
    wv = din("wv", [512, 256])
    wvb = din("wvb", [1, 256])
    bq2 = din("bq2", [128, 2], f32)
    bk2 = din("bk2", [128, 2], f32)
    projT = din("projT", [128, 256])   # proj.T stacked twice
    bd = din("bd", [128, 8])
    msk = din("msk", [128, 512])       # [tri | ones | zeros | tri]
    wo2 = din("wo2", [256, 512])
    outT = nc.dram_tensor("outT", [512, L], bf16, kind="ExternalOutput").ap()
    dbg_att = None
    if _DEBUG_ATT:
        dbg_att = [nc.dram_tensor(f"dbg_att{i}", [128, L], bf16,
                                  kind="ExternalOutput").ap()
                   for i in range(2)]

    with TileContext(nc) as tc, nc.allow_low_precision(
            reason="bf16 scan pipeline, validated 4.8e-3 rel err vs f32 "
                   "reference (tolerance 2e-2)"):
        with (
            tc.tile_pool(name="const", bufs=1) as cpool,
            tc.tile_pool(name="persist", bufs=1) as ppool,
            tc.tile_pool(name="psP", bufs=2, space="PSUM") as psP,
        ):
            # ---- constants ----
            def ldconst(name, shape, src, dt=bf16):
                t = cpool.tile(shape, dt, name=name, tag=name)
                nc.sync.dma_start(t[:, :], src)
                return t

            c_projT = ldconst("projT", [128, 256], projT)
            c_bd = ldconst("bd", [128, 8], bd)
            c_msk = ldconst("msk", [128, 512], msk)
            c_bq = ldconst("bq", [128, 2], bq2, f32)
            c_bk = ldconst("bk", [128, 2], bk2, f32)
            c_wq = [ldconst(f"wq{i}", [128, 256], wq[128 * i:128 * (i + 1), :])
                    for i in range(4)]
            c_wk = [ldconst(f"wk{i}", [128, 256], wk[128 * i:128 * (i + 1), :])
                    for i in range(4)]
            c_wv = [ldconst(f"wv{i}", [128, 256], wv[128 * i:128 * (i + 1), :])
                    for i in range(4)]
            c_wvb = ldconst("wvb", [1, 256], wvb)
            c_wo2 = [ldconst(f"wo2{i}", [128, 512],
                             wo2[128 * i:128 * (i + 1), :]) for i in range(2)]
            c_cneg = cpool.tile([128, 128], bf16, tag="cneg")
            nc.any.memset(c_cneg[:, :], -0.5)
            c_ones = cpool.tile([1, 128], bf16, tag="ones")
            nc.any.memset(c_ones[:, :], 1.0)
            c_zS = cpool.tile([128, 132], bf16, tag="zS")
            nc.any.memset(c_zS[:, :], 0.0)

            # persistent activations
            t_qT = [ppool.tile([128, L], bf16, name=f"qT{i}", tag=f"qT{i}")
                    for i in range(2)]
            t_kT = [ppool.tile([128, L], bf16, name=f"kT{i}", tag=f"kT{i}")
                    for i in range(2)]
            t_sq = [ppool.tile([128, L], bf16, name=f"sq{i}", tag=f"sq{i}")
                    for i in range(2)]
            t_v = ppool.tile([128, NC2 * 264], bf16, tag="vall")
            t_ksc = ppool.tile([128, NC2 * 4], f32, tag="ksc")

            # ---- Phase 1: x loads + projections ----
            xin = tc.tile_pool(name="xin", bufs=1)
            xpool = xin.__enter__()
            t_x = {}
            for nm, src in (("q", xq), ("k", xk), ("v", xv)):
                for i in range(4):
                    xt = xpool.tile([128, L], bf16, name=f"x{nm}{i}",
                                    tag=f"x{nm}{i}")
                    nc.sync.dma_start(xt[:, :], src[128 * i:128 * (i + 1), :])
                    t_x[(nm, i)] = xt

            for (wgt, nm, dst, bias) in ((c_wq, "q", t_qT, c_bq),
                                         (c_wk, "k", t_kT, c_bk)):
                for half in range(2):
                    for lt in range(NLT):
                        ls = slice(lt * LT, (lt + 1) * LT)
                        ps = psP.tile([128, LT], f32, tag="psP")
                        for kt in range(4):
                            nc.tensor.matmul(
                                ps[:, :],
                                wgt[kt][:, 128 * half:128 * (half + 1)],
                                t_x[(nm, kt)][:, ls],
                                start=(kt == 0), stop=(kt == 3))
                        nc.scalar.add(dst[half][:, ls], ps[:, :],
                                      bias[:, half:half + 1])

            # sq = kT^2 (Pool engine; SBUF-only op)
            for half in range(2):
                nc.gpsimd.tensor_mul(t_sq[half][:, :], t_kT[half][:, :],
                                     t_kT[half][:, :])

            # ksc[l, 4ch+h] = -0.5 * sum_d kT^2  (per-chunk, per-head)
            for ch in range(NC2):
                cs = slice(ch * 128, (ch + 1) * 128)
                ps = psP.tile([128, LT], f32, tag="psP")
                for half in range(2):
                    nc.tensor.matmul(ps[:, 0:4], t_sq[half][:, cs],
                                     c_bd[:, 4 * half:4 * (half + 1)],
                                     start=(half == 0), stop=(half == 1))
                nc.scalar.copy(t_ksc[:, 4 * ch:4 * (ch + 1)], ps[:, 0:4])

            # v projection (chunked [l, 4h x 66] layout with ones columns)
            v_r4 = t_v[:, :].rearrange("p (c h x) -> p c h x", h=4, x=66)
            nc.any.memset(v_r4[:, :, :, 64:66], 1.0)
            for ch in range(NC2):
                cs = slice(ch * 128, (ch + 1) * 128)
                ps = psP.tile([128, LT], f32, tag="psP")
                for kt in range(4):
                    nc.tensor.matmul(ps[:, 0:256], t_x[("v", kt)][:, cs],
                                     c_wv[kt][:, :],
                                     start=(kt == 0), stop=False)
                nc.tensor.matmul(ps[:, 0:256], c_ones[0:1, 0:128],
                                 c_wvb[:, :], start=False, stop=True)
                ps_r = ps[:, 0:256].rearrange("p (h x) -> p h x", h=4)
                nc.scalar.copy(v_r4[:, ch, :, 0:64], ps_r[:, :, :])
            xin.__exit__(None, None, None)

            # ---- head pipeline ----
            hctx = (tc.tile_pool(name="headbuf", bufs=2),
                    tc.tile_pool(name="work", bufs=3),
                    tc.tile_pool(name="attn", bufs=1))
            pctx = (tc.tile_pool(name="psA", bufs=2, space="PSUM"),
                    tc.tile_pool(name="psS", bufs=2, space="PSUM"),
                    tc.tile_pool(name="psND", bufs=2, space="PSUM"))
            hpool, wpool, apool = [c.__enter__() for c in hctx]
            psA_p, psS_p, psND_p = [c.__enter__() for c in pctx]

            t_att = [apool.tile([128, L], bf16, name=f"att{i}", tag=f"att{i}")
                     for i in range(2)]

            heads = {}

            def gen(h):
                hh = h // 2
                hr = slice(64 * (h % 2), 64 * (h % 2) + 64)
                pr = hr
                t_qp = [hpool.tile([128, L], bf16, name=f"qp{i}",
                                   tag=f"qp{i}") for i in range(2)]
                t_kp = [hpool.tile([128, L], bf16, name=f"kp{i}",
                                   tag=f"kp{i}") for i in range(2)]
                t_kplc = hpool.tile([128, NC2 * 256], bf16, tag="kplc")
                t_S = hpool.tile([128, (NCC - 1) * 132], bf16, tag="S")
                for half in range(2):
                    mh = slice(128 * half, 128 * (half + 1))
                    for lt in range(NLT):
                        ls = slice(lt * LT, (lt + 1) * LT)
                        ps = psP.tile([128, LT], f32, tag="psP")
                        nc.tensor.matmul(ps[:, :], c_projT[pr, mh],
                                         t_qT[hh][hr, ls],
                                         start=True, stop=True)
                        nc.scalar.activation(t_qp[half][:, ls], ps[:, :], EXP)
                        ps2 = psP.tile([128, LT], f32, tag="psP")
                        nc.tensor.matmul(ps2[:, :], c_projT[pr, mh],
                                         t_kT[hh][hr, ls],
                                         start=True, stop=False)
                        nc.tensor.matmul(ps2[:, :], c_cneg[pr, :],
                                         t_sq[hh][hr, ls],
                                         start=False, stop=True)
                        nc.scalar.activation(t_kp[half][:, ls], ps2[:, :], EXP)
                for ch in range(NC2):
                    cs = slice(ch * 128, (ch + 1) * 128)
                    ps = psP.tile([128, LT], f32, tag="psP")
                    nc.tensor.matmul(ps[:, 0:256], t_kT[hh][hr, cs],
                                     c_projT[pr, :], start=True, stop=True)
                    nc.scalar.activation(
                        t_kplc[:, 256 * ch:256 * (ch + 1)], ps[:, 0:256], EXP,
                        bias=t_ksc[:, 4 * ch + h:4 * ch + h + 1])
                heads[h] = (t_qp, t_kp, t_kplc, t_S)

            def scan(h):
                t_qp, t_kp, t_kplc, t_S = heads[h]
                arow = slice(64 * (h % 2), 64 * (h % 2) + 64)
                att = t_att[h // 2]

                def va(c128):
                    o = c128 * 264 + h * 66
                    return t_v[:, o:o + 66]

                # state sums + prefix snapshots
                psS_t = {}
                for cc in range(NCC):
                    c0, c1 = 2 * cc, 2 * cc + 1
                    psS = psS_p.tile([128, 132], f32, tag="psS")
                    for mh in range(2):
                        r = slice(66 * mh, 66 * mh + 66)
                        nc.tensor.matmul(
                            psS[:, r],
                            t_kplc[:, c0 * 256 + 128 * mh:
                                   c0 * 256 + 128 * mh + 128],
                            va(c0), start=(mh == 0), stop=False)
                        nc.tensor.matmul(
                            psS[:, r],
                            t_kplc[:, c1 * 256 + 128 * mh:
                                   c1 * 256 + 128 * mh + 128],
                            va(c1), start=False, stop=(mh == 1))
                    psS_t[cc] = psS
                    if cc == 0:
                        continue
                    dst = t_S[:, (cc - 1) * 132:cc * 132]
                    if cc == 1:
                        nc.vector.tensor_copy(dst, psS_t[0][:, :])
                    else:
                        nc.vector.tensor_add(
                            dst, t_S[:, (cc - 2) * 132:(cc - 1) * 132],
                            psS_t[cc - 1][:, :])

                # chunk loop, psA/mask software-pipelined one chunk ahead
                psA_t, atm_t = {}, {}

                def emit_psA(cc):
                    qs = slice(cc * CW, (cc + 1) * CW)
                    klo = slice(cc * CW, cc * CW + 128)
                    khi = slice(cc * CW + 128, (cc + 1) * CW)
                    # single start..stop bracket per bank: start lazily
                    # zeroes the whole 2KB region, interleaved brackets
                    # clobber sibling regions
                    psA = psA_p.tile([128, 512], f32, tag="psA")
                    nc.tensor.matmul(psA[:, 0:256], t_kp[0][:, klo],
                                     t_qp[0][:, qs], start=True, stop=False)
                    nc.tensor.matmul(psA[:, 0:256], t_kp[1][:, klo],
                                     t_qp[1][:, qs], start=False, stop=False)
                    nc.tensor.matmul(psA[:, 256:512], t_kp[0][:, khi],
                                     t_qp[0][:, qs], start=False, stop=False)
                    nc.tensor.matmul(psA[:, 256:512], t_kp[1][:, khi],
                                     t_qp[1][:, qs], start=False, stop=True)
                    psA_t[cc] = psA

                def emit_mask(cc):
                    atm = wpool.tile([128, 512], bf16, tag="atm")
                    nc.vector.tensor_mul(atm[:, :], psA_t[cc][:, :],
                                         c_msk[:, :])
                    atm_t[cc] = atm

                emit_psA(0)
                emit_psA(1)
                emit_mask(0)
                for cc in range(NCC):
                    qs = slice(cc * CW, (cc + 1) * CW)
                    if cc + 2 < NCC:
                        emit_psA(cc + 2)
                    if cc + 1 < NCC:
                        emit_mask(cc + 1)
                    c0, c1 = 2 * cc, 2 * cc + 1
                    atm = atm_t.pop(cc)
                    psA_t.pop(cc, None)
                    S_src = c_zS if cc == 0 else t_S[:, (cc - 1) * 132:cc * 132]
                    # full-bank tile: nd in [0:66, 0:256], den-reciprocal
                    # broadcast parked in the spare quadrant [64:128, 256:512]
                    nd = psND_p.tile([128, 512], f32, tag="psND")
                    nc.tensor.matmul(nd[0:66, 0:256], S_src[:, 0:66],
                                     t_qp[0][:, qs], start=True, stop=False)
                    nc.tensor.matmul(nd[0:66, 0:256], S_src[:, 66:132],
                                     t_qp[1][:, qs], start=False, stop=False)
                    nc.tensor.matmul(nd[0:66, 0:256], va(c1),
                                     atm[:, 256:512], start=False, stop=False)
                    nc.tensor.matmul(nd[0:66, 0:256], va(c0),
                                     atm[:, 0:256], start=False, stop=True)
                    t_rcp = wpool.tile([1, 256], bf16, tag="rcp")
                    nc.vector.reciprocal(t_rcp[:, :], nd[64:65, 0:256])
                    nc.tensor.matmul(nd[64:128, 256:512], c_ones[0:1, 0:64],
                                     t_rcp[:, :], start=True, stop=True)
                    rcpB = wpool.tile([64, 256], bf16, tag="rcpB")
                    nc.vector.tensor_copy(rcpB[:, :], nd[64:128, 256:512])
                    nc.vector.tensor_mul(att[arow, qs], nd[0:64, 0:256],
                                         rcpB[:, :])

            # ---- emission schedule: gen runs ahead of scan by one head ----
            gen(0)
            gen(1)
            scan(0)
            gen(2)
            scan(1)
            gen(3)
            scan(2)
            scan(3)
            for c in reversed(pctx):
                c.__exit__(None, None, None)
            if _DEBUG_ATT:
                for i in range(2):
                    nc.sync.dma_start(dbg_att[i], t_att[i][:, :])

            # ---- output projection ----
            octx = tc.tile_pool(name="outp", bufs=2)
            opool = octx.__enter__()
            psO_c = tc.tile_pool(name="psO", bufs=2, space="PSUM")
            psO_p = psO_c.__enter__()
            for osub in range(4):
                os_ = slice(128 * osub, 128 * (osub + 1))
                for lt in range(NLT):
                    ls = slice(lt * LT, (lt + 1) * LT)
                    ps = psO_p.tile([128, LT], f32, tag="psO")
                    nc.tensor.matmul(ps[:, :], c_wo2[0][:, os_],
                                     t_att[0][:, ls], start=True, stop=False)
                    nc.tensor.matmul(ps[:, :], c_wo2[1][:, os_],
                                     t_att[1][:, ls], start=False, stop=True)
                    t_o = opool.tile([128, LT], bf16, tag="outT")
                    if lt % 2 == 0:
                        nc.scalar.copy(t_o[:, :], ps[:, :])
                    else:
                        nc.vector.tensor_copy(t_o[:, :], ps[:, :])
                    nc.sync.dma_start(outT[os_, ls], t_o[:, :])
            psO_c.__exit__(None, None, None)
            octx.__exit__(None, None, None)
            for c in reversed(hctx):
                c.__exit__(None, None, None)

    nc.compile()
    return nc


def _prep_inputs(query, key, value, Wq, bq, Wk, bk, Wv, bv, Wo, bo, proj):
    from ml_dtypes import bfloat16
    s = float(DK) ** -0.25

    def bf(x):
        return np.ascontiguousarray(x).astype(bfloat16)

    tri = (np.arange(128)[:, None] <= np.arange(128)[None, :]).astype(
        np.float32)
    on = np.ones((128, 128), np.float32)
    zr = np.zeros((128, 128), np.float32)
    msk = np.concatenate([tri, on, zr, tri], axis=1)
    bd = np.zeros((128, 8), np.float32)
    for half in range(2):
        for r in range(128):
            bd[r, 4 * half + (2 * half + r // 64)] = -0.5
    pT = np.ascontiguousarray(proj.T)
    common = {"projT": bf(np.concatenate([pT, pT])), "bd": bf(bd),
              "msk": bf(msk)}
    in_maps = []
    for b in range(B):
        for hg in range(2):
            sl = slice(hg * 256, (hg + 1) * 256)
            m = dict(common)
            m["xq"] = bf(query[b].T)
            m["xk"] = bf(key[b].T)
            m["xv"] = bf(value[b].T)
            m["wq"] = bf(Wq[sl].T * s)
            m["wk"] = bf(Wk[sl].T * s)
            m["wv"] = bf(Wv[sl].T)
            m["wvb"] = bf(bv[sl][None, :])
            m["bq2"] = np.ascontiguousarray(
                np.stack([bq[sl][:128] * s, bq[sl][128:] * s], axis=1),
                np.float32)
            m["bk2"] = np.ascontiguousarray(
                np.stack([bk[sl][:128] * s, bk[sl][128:] * s], axis=1),
                np.float32)
            m["wo2"] = bf(Wo[:, sl].T)
            in_maps.append(m)
    return in_maps


def kernel(query, key, value, Wq, bq, Wk, bk, Wv, bv, Wo, bo, proj,
           _trace=False):
    global _COMPILED
    from concourse import bass_utils
    args = [np.asarray(a, np.float32) for a in
            (query, key, value, Wq, bq, Wk, bk, Wv, bv, Wo, bo, proj)]
    if _COMPILED is None:
        _COMPILED = _build()
    in_maps = _prep_inputs(*args)
    res = bass_utils.run_bass_kernel_spmd(
        _COMPILED, in_maps, core_ids=list(range(8)), trace=_trace)
    out = np.empty((B, L, DIM), np.float32)
    bo_ = args[10]
    for b in range(B):
        out[b] = (res.results[2 * b]["outT"].astype(np.float32).T
                  + res.results[2 * b + 1]["outT"].astype(np.float32).T + bo_)
    if _trace:
        kernel._last = res
    return out
